# revision 15
# baseline (speedup 1.0000x reference)
"""Causal self-attention (B=4, T=2048, E=512, H=8) on 8 TRN2 NeuronCores.

Sharding: core c -> (batch b = c//2, head-group hg = c%2, 4 heads each).
Each core computes qkv projections for its 4 heads over the full sequence,
flash-style causal attention (S^T layout, softmax sums via an ones-column
appended to V), and a partial output projection (256 of the 512 contraction
rows). Host sums the two partials per batch (the tensor-parallel all-reduce)
— c_proj bias is supplied to the hg==0 core only.

Compute dtype is fp16 (PE streams 1 col/cycle vs 4 for fp32); accumulation
and softmax normalization stay fp32. x is supplied pre-transposed
(feature-major) by the host, so no on-chip input transpose is needed.
"""

from contextlib import ExitStack

import numpy as np

import concourse.bass as bass
import concourse.mybir as mybir
import concourse.tile as tile
from concourse import bacc
from concourse.bass import ts
from concourse.bass_utils import run_bass_kernel_spmd

f32 = mybir.dt.float32
f16 = mybir.dt.float16
FA = mybir.ActivationFunctionType

B, T, E = 4, 2048, 512
H, D = 8, 64
HPC = 4              # heads per core
EC = HPC * D         # 256: per-core qkv width per projection
P = 128
NCORES = 8
TQ = T // P          # 16 query/token chunks
NQG = T // 512       # 4 query groups of 512
EO = E // P          # 4 contraction subtiles for E
SCALE = 1.0 / np.sqrt(D)


def _emit(tc, ctx, aps, reps=1):
    nc = tc.nc
    z = aps["z"]

    cst = ctx.enter_context(tc.tile_pool(name="cst", bufs=1))
    wqk_sb = cst.tile([P, EO, 2 * EC], f16)
    for eo in range(EO):
        nc.sync.dma_start(wqk_sb[:, eo, :], aps["wqk"][:, eo, :])
    bqk_sb = cst.tile([P, 4], f32)
    nc.sync.dma_start(bqk_sb, aps["bqk"])
    wv_sb = cst.tile([P, EO, EC], f16)
    nc.sync.dma_start(wv_sb, aps["wv"])
    bv_sb = cst.tile([1, EC], f16)
    nc.sync.dma_start(bv_sb, aps["bv"])
    wp_sb = cst.tile([P, 2, E], f16)
    nc.sync.dma_start(wp_sb, aps["wp"])
    bp_sb = cst.tile([1, E], f16)
    nc.sync.dma_start(bp_sb, aps["bp"])
    ident16_sb = cst.tile([P, P], f16)
    nc.sync.dma_start(ident16_sb, aps["ident16"])
    ones_sb = cst.tile([1, P], f16)
    nc.sync.dma_start(ones_sb, aps["ones"])
    mask_sb = cst.tile([P, 4, 512], f16)  # 0/1 causal staircase for diag blocks
    nc.sync.dma_start(mask_sb, aps["mask"])

    big = ctx.enter_context(tc.tile_pool(name="big", bufs=1))
    qkT = big.tile([P, EO, T], f16)          # rows 0-255: q^T, 256-511: k^T
    v_sb = big.tile([P, TQ, HPC * 65], f16)  # per head: 64 v cols + ones col
    yT = big.tile([P, 2, T], f16)            # attention output, feature-major
    v4 = v_sb.rearrange("p t (h c) -> p t h c", c=65)

    # psum: pS 2x[128,1536] (6 banks) for score groups; pM 2x[128,512]
    # (2 banks) for everything else.
    pS = ctx.enter_context(tc.tile_pool(name="pS", bufs=2, space="PSUM"))
    pM = ctx.enter_context(tc.tile_pool(name="pM", bufs=2, space="PSUM"))
    yn_p = ctx.enter_context(tc.tile_pool(name="yn_p", bufs=2))
    rc_p = ctx.enter_context(tc.tile_pool(name="rc_p", bufs=2))
    zout = ctx.enter_context(tc.tile_pool(name="zout", bufs=3))

    for _ in range(reps):
        with tc.tile_pool(name="xTp", bufs=1) as xTp:
            xT = xTp.tile([P, EO, T], f16)
            for eo in range(EO):
                for th in range(2):
                    nc.sync.dma_start(
                        xT[:, eo, ts(th, T // 2)], aps["xT"][:, eo, ts(th, T // 2)]
                    )

            # ---- phase 2: q^T,k^T = (x @ Wqk + b)^T (feature-major) ----
            for jc in range(EO):
                for tg in range(4):
                    ps = pM.tile([P, 512], f32, tag="w")
                    for eo in range(EO):
                        nc.tensor.matmul(
                            ps,
                            lhsT=wqk_sb[:, eo, ts(jc, P)],
                            rhs=xT[:, eo, ts(tg, 512)],
                            start=(eo == 0),
                            stop=(eo == EO - 1),
                        )
                    nc.vector.tensor_scalar_add(
                        qkT[:, jc, ts(tg, 512)], ps, bqk_sb[:, jc : jc + 1]
                    )

            # ---- phase 3: v = x @ Wv + b (token-major), plus ones col ----
            nc.vector.memset(v4[:, :, :, 64], 1.0)
            for tq in range(TQ):
                pv = pM.tile([P, 512], f32, tag="w")
                for eo in range(EO):
                    nc.tensor.matmul(
                        pv[:, :EC],
                        lhsT=xT[:, eo, ts(tq, P)],
                        rhs=wv_sb[:, eo, :],
                        start=(eo == 0),
                        stop=False,
                    )
                nc.tensor.matmul(
                    pv[:, :EC], lhsT=ones_sb, rhs=bv_sb, start=False, stop=True
                )
                nc.vector.tensor_copy(
                    v4[:, tq, :, 0:64], pv[:, :EC].rearrange("p (h c) -> p h c", c=64)
                )

        # ---- phase 4+5: attention per query group / head, then that
        # group's slice of the output projection ----
        with tc.tile_pool(name="expSp", bufs=2) as expSp:
            for qg in range(NQG):
                for h in range(HPC):
                    hp = (h % 2) * 64    # partition offset of this head's dims
                    q_sub = h // 2       # q rows live in subtile h//2
                    k_sub = 2 + h // 2   # k rows live in subtile 2 + h//2
                    expS = expSp.tile([P, TQ, 512], f16, tag="e")
                    # S^T in groups of up to 3 kt-blocks -> one exp per group
                    nb = 4 * qg + 4
                    for g0 in range(0, nb, 3):
                        gw = min(3, nb - g0)
                        pSt = pS.tile([P, 3 * 512], f32, tag="s")
                        for j in range(gw):
                            kb = g0 + j
                            nc.tensor.matmul(
                                pSt[:, ts(j, 512)],
                                lhsT=qkT[hp : hp + 64, k_sub, ts(kb, P)],
                                rhs=qkT[hp : hp + 64, q_sub, ts(qg, 512)],
                                start=True,
                                stop=True,
                            )
                        nc.scalar.activation(
                            expS[:, g0 : g0 + gw, :],
                            pSt[:, : gw * 512].rearrange("p (b c) -> p b c", c=512),
                            FA.Exp,
                        )
                    # zero causally-invalid region of the 4 diagonal blocks
                    # (0/1 staircase mask, DVE 4x fp16 mode)
                    nc.vector.tensor_tensor(
                        expS[:, 4 * qg : 4 * qg + 4, :],
                        expS[:, 4 * qg : 4 * qg + 4, :],
                        mask_sb,
                        mybir.AluOpType.mult,
                    )
                    # PV: y[qt, 0:64] + row-sum in col 64, for 4 query chunks
                    py = pM.tile([P, 512], f32, tag="w")
                    for qc_l in range(4):
                        qc = 4 * qg + qc_l
                        for kb in range(qc + 1):
                            nc.tensor.matmul(
                                py[:, qc_l * 65 : (qc_l + 1) * 65],
                                lhsT=expS[:, kb, ts(qc_l, P)],
                                rhs=v_sb[:, kb, h * 65 : (h + 1) * 65],
                                start=(kb == 0),
                                stop=(kb == qc),
                            )
                    pyv = py[:, : 4 * 65].rearrange("p (q c) -> p q c", c=65)
                    rc = rc_p.tile([P, 4], f32, tag="r")
                    nc.vector.reciprocal(rc, pyv[:, :, 64])
                    yn = yn_p.tile([P, 4, 64], f16, tag="n")
                    nc.vector.tensor_tensor(
                        yn,
                        pyv[:, :, 0:64],
                        rc[:, :, None].to_broadcast((P, 4, 64)),
                        mybir.AluOpType.mult,
                    )
                    ptr = pM.tile([P, 512], f16, tag="w")
                    for qc_l in range(4):
                        nc.tensor.transpose(
                            ptr[0:64, ts(qc_l, P)], yn[:, qc_l, :], ident16_sb
                        )
                    nc.vector.tensor_copy(
                        yT[hp : hp + 64, h // 2, ts(qg, 512)], ptr[0:64, :]
                    )

                # z = y @ Wp + bias for this query group's 4 token chunks
                for tq in range(4 * qg, 4 * qg + 4):
                    pz = pM.tile([P, 512], f32, tag="w")
                    for eo in range(2):
                        nc.tensor.matmul(
                            pz,
                            lhsT=yT[:, eo, ts(tq, P)],
                            rhs=wp_sb[:, eo, :],
                            start=(eo == 0),
                            stop=False,
                        )
                    nc.tensor.matmul(
                        pz, lhsT=ones_sb, rhs=bp_sb, start=False, stop=True
                    )
                    zt = zout.tile([P, E], f16, tag="z")
                    nc.vector.tensor_copy(zt, pz)
                    nc.sync.dma_start(z[ts(tq, P), :], zt)


def build(reps=1):
    nc = bacc.Bacc("TRN2", target_bir_lowering=False, debug=False)
    aps = {
        "xT": nc.dram_tensor("xT", [P, EO, T], f16, kind="ExternalInput").ap(),
        "wqk": nc.dram_tensor("wqk", [P, EO, 2 * EC], f16, kind="ExternalInput").ap(),
        "bqk": nc.dram_tensor("bqk", [P, 4], f32, kind="ExternalInput").ap(),
        "wv": nc.dram_tensor("wv", [P, EO, EC], f16, kind="ExternalInput").ap(),
        "bv": nc.dram_tensor("bv", [1, EC], f16, kind="ExternalInput").ap(),
        "wp": nc.dram_tensor("wp", [P, 2, E], f16, kind="ExternalInput").ap(),
        "bp": nc.dram_tensor("bp", [1, E], f16, kind="ExternalInput").ap(),
        "ident16": nc.dram_tensor("ident16", [P, P], f16, kind="ExternalInput").ap(),
        "ones": nc.dram_tensor("ones", [1, P], f16, kind="ExternalInput").ap(),
        "mask": nc.dram_tensor("mask", [P, 4, 512], f16, kind="ExternalInput").ap(),
        "z": nc.dram_tensor("z", [T, E], f16, kind="ExternalOutput").ap(),
    }
    with tile.TileContext(nc) as tc, ExitStack() as ctx:
        _emit(tc, ctx, aps, reps=reps)
    nc.compile()
    return nc


def make_in_maps(x, c_attn_w, c_attn_b, c_proj_w, c_proj_b):
    x = np.asarray(x, np.float32)
    W = np.asarray(c_attn_w, np.float32)
    bW = np.asarray(c_attn_b, np.float32)
    Wp = np.asarray(c_proj_w, np.float32)
    bp = np.asarray(c_proj_b, np.float32)

    ident16 = np.eye(P, dtype=np.float16)
    ones = np.ones((1, P), np.float16)
    # mask[p, j, c] = 1 iff query col c >= key row p + 128*j (causal staircase)
    pp = np.arange(P)[:, None, None]
    jj = np.arange(4)[None, :, None]
    cc = np.arange(512)[None, None, :]
    mask = (cc >= pp + 128 * jj).astype(np.float16)
    in_maps = []
    for c in range(NCORES):
        b, hg = c // 2, c % 2
        qs = slice(hg * EC, (hg + 1) * EC)
        ks = slice(E + hg * EC, E + (hg + 1) * EC)
        vs = slice(2 * E + hg * EC, 2 * E + (hg + 1) * EC)
        wqk = np.concatenate([W[:, qs] * SCALE, W[:, ks]], axis=1)  # [512, 512]
        bqk = np.concatenate([bW[qs] * SCALE, bW[ks]])              # [512]
        # x^T in [p, eo, t] layout: xT[p, eo, t] = x[t, eo*128 + p]
        xT = np.ascontiguousarray(
            x[b].T.reshape(EO, P, T).transpose(1, 0, 2)
        ).astype(np.float16)
        in_maps.append({
            "xT": xT,
            "wqk": np.ascontiguousarray(
                wqk.reshape(EO, P, 2 * EC).transpose(1, 0, 2)
            ).astype(np.float16),
            "bqk": np.ascontiguousarray(bqk.reshape(4, P).T),
            "wv": np.ascontiguousarray(
                W[:, vs].reshape(EO, P, EC).transpose(1, 0, 2)
            ).astype(np.float16),
            "bv": bW[vs][None].astype(np.float16),
            "wp": np.ascontiguousarray(
                Wp[hg * EC : (hg + 1) * EC, :].reshape(2, P, E).transpose(1, 0, 2)
            ).astype(np.float16),
            "bp": ((bp if hg == 0 else np.zeros_like(bp))[None]).astype(np.float16),
            "ident16": ident16,
            "ones": ones,
            "mask": mask,
        })
    return in_maps


_NC_CACHE = {}


def kernel(x, c_attn_w, c_attn_b, c_proj_w, c_proj_b):
    if "nc" not in _NC_CACHE:
        _NC_CACHE["nc"] = build()
    nc = _NC_CACHE["nc"]
    in_maps = make_in_maps(x, c_attn_w, c_attn_b, c_proj_w, c_proj_b)
    res = run_bass_kernel_spmd(nc, in_maps, core_ids=list(range(NCORES)))
    out = np.empty((B, T, E), np.float32)
    for b in range(B):
        out[b] = (
            res.results[2 * b]["z"].astype(np.float32)
            + res.results[2 * b + 1]["z"].astype(np.float32)
        )
    return out
